# revision 7
# baseline (speedup 1.0000x reference)
"""Multi-head attention (b=4, n=2048, embed=768, heads=8) on 8 TRN2 NeuronCores.

Sharding: tensor-parallel over heads — one head per core. Each core computes
its head's Q^T/K^T/V^T projections from the full token stream, runs attention
in a fully "transposed" layout (softmax over the free dim, no attention-matrix
transposes), then a per-batch AllToAll redistributes per-head outputs so each
core owns a 256-token slice of every batch and computes the final projection
(bias folded in via an all-ones lhsT row).

All matmuls run as float32r (4-byte fp32 operands on the fast weight-load
path; ~tf32-class accuracy, ~3x the fp32 matmul rate).
"""

import numpy as np

import concourse.bass as bass
import concourse.tile as tile
from concourse import bacc, mybir
from concourse.bass_utils import run_bass_kernel_spmd
from concourse.masks import make_identity

F32 = mybir.dt.float32
F32R = mybir.dt.float32r

P = 128
EMB = 768
D = 96          # head dim
DP = 97         # head dim + denominator/ones row
NB = 4          # batches
SEQ = 2048      # tokens per batch
QW = 512        # q window width
NWIN = SEQ // QW        # q windows per batch (4)
NCHUNK = SEQ // P       # k chunks per batch (16)
NCORES = 8
NSLOT = SEQ // NCORES   # tokens per (batch, core) output slot (256)
ECHUNK = EMB // P       # embed chunks (6)

_CACHED_NC = None


def _build_nc():
    nc = bacc.Bacc(None, num_devices=NCORES)

    xT = nc.declare_dram_parameter("xT", [EMB, NB * SEQ], F32, isOutput=False)
    wq = nc.declare_dram_parameter("wq", [EMB, D], F32, isOutput=False)
    wk = nc.declare_dram_parameter("wk", [EMB, D], F32, isOutput=False)
    wv = nc.declare_dram_parameter("wv", [EMB, D], F32, isOutput=False)
    bqkv = nc.declare_dram_parameter("bqkv", [D, 3], F32, isOutput=False)
    wp = nc.declare_dram_parameter("wp", [NCORES, DP, EMB], F32, isOutput=False)
    out = nc.declare_dram_parameter("out", [NB, NSLOT, EMB], F32, isOutput=True)

    o_send = nc.dram_tensor("o_send", [NB, NCORES, DP, NSLOT], F32)
    o_recv = nc.dram_tensor("o_recv", [NB, NCORES, DP, NSLOT], F32)

    groups = [list(range(NCORES))]

    with tile.TileContext(nc) as tc:
        with (
            tc.tile_pool(name="singles", bufs=1) as singles,
            tc.tile_pool(name="qkvT", bufs=2) as qkvT_pool,
            tc.tile_pool(name="xwin", bufs=3) as xwin_pool,
            tc.tile_pool(name="vp", bufs=2) as vp_pool,
            tc.tile_pool(name="at", bufs=4) as at_pool,
            tc.tile_pool(name="bc", bufs=2) as bc_pool,
            tc.tile_pool(name="ot", bufs=3) as ot_pool,
            tc.tile_pool(name="po", bufs=2) as po_pool,
            tc.tile_pool(name="lh", bufs=10) as lh_pool,
            tc.tile_pool(name="psA", bufs=2, space="PSUM") as psA,
            tc.tile_pool(name="psS", bufs=2, space="PSUM") as psS,
            tc.tile_pool(name="psO", bufs=1, space="PSUM") as psO,
            tc.tile_pool(name="psP", bufs=1, space="PSUM") as psP,
        ):
            # ---- constants ----
            wq_sb = singles.tile([P, ECHUNK, D], F32R)
            wk_sb = singles.tile([P, ECHUNK, D], F32R)
            wv_sb = singles.tile([P, ECHUNK, D], F32R)
            for wsb, wdr in ((wq_sb, wq), (wk_sb, wk), (wv_sb, wv)):
                nc.sync.dma_start(
                    out=wsb, in_=wdr[:, :].bitcast(F32R).rearrange("(c p) d -> p c d", p=P))
            bias_sb = singles.tile([D, 3], F32)
            nc.sync.dma_start(out=bias_sb, in_=bqkv[:, :])
            wp_sb = singles.tile([DP, NCORES, EMB], F32R)
            nc.sync.dma_start(out=wp_sb, in_=wp[:, :, :].bitcast(F32R).rearrange("h p e -> p h e"))
            ident = singles.tile([D, D], F32)
            make_identity(nc, ident[:, :])
            ones_sb = singles.tile([P, 1], F32)
            nc.vector.memset(ones_sb[:, :], 1.0)

            for bb in range(NB):
                # ---- QKV projections for this batch (transposed layout) ----
                qT = qkvT_pool.tile([D, SEQ], F32R, tag="qT")
                kT = qkvT_pool.tile([D, SEQ], F32R, tag="kT")
                vT = qkvT_pool.tile([D, SEQ], F32R, tag="vT")
                for w in range(NWIN):
                    col0 = bb * SEQ + w * QW
                    xt = xwin_pool.tile([P, ECHUNK, QW], F32R)
                    nc.sync.dma_start(
                        out=xt,
                        in_=xT[:, col0:col0 + QW].bitcast(F32R).rearrange(
                            "(c p) t -> p c t", p=P))
                    for dstT, wsb, bcol in (
                        (qT, wq_sb, 0), (kT, wk_sb, 1), (vT, wv_sb, 2),
                    ):
                        ps = psA.tile([D, QW], F32, tag="a")
                        for c in range(ECHUNK):
                            nc.tensor.matmul(
                                ps,
                                lhsT=wsb[:, c, :],
                                rhs=xt[:, c, :],
                                start=(c == 0), stop=(c == ECHUNK - 1))
                        nc.vector.tensor_scalar_add(
                            out=dstT[:, w * QW:(w + 1) * QW],
                            in0=ps,
                            scalar1=bias_sb[:, bcol:bcol + 1])

                # ---- V' = [V | ones] per 128-token chunk (PE transpose) ----
                vP = vp_pool.tile([P, NCHUNK, DP], F32R)
                nc.vector.tensor_copy(
                    out=vP[:, :, 0:1],
                    in_=ones_sb[:, 0:1].to_broadcast((P, NCHUNK, 1)))
                for k in range(NCHUNK):
                    pst = psA.tile([P, D], F32, tag="a")
                    nc.tensor.transpose(
                        out=pst, in_=vT[:, k * P:(k + 1) * P].bitcast(F32),
                        identity=ident[:, :])
                    nc.vector.tensor_copy(out=vP[:, k, 1:DP], in_=pst)

                # ---- attention, one q-window at a time ----
                for w in range(NWIN):
                    qs = slice(w * QW, (w + 1) * QW)
                    ps_o = psO.tile([DP, QW], F32)
                    for kk in range(NCHUNK // 2):
                        ps_s = psS.tile([P, 2, QW], F32)
                        for j in range(2):
                            k = 2 * kk + j
                            nc.tensor.matmul(
                                ps_s[:, j, :],
                                lhsT=kT[:, k * P:(k + 1) * P],
                                rhs=qT[:, qs],
                                start=True, stop=True)
                        aT = at_pool.tile([P, 2, QW], F32R)
                        nc.scalar.activation(
                            out=aT, in_=ps_s,
                            func=mybir.ActivationFunctionType.Exp)
                        for j in range(2):
                            k = 2 * kk + j
                            nc.tensor.matmul(
                                ps_o,
                                lhsT=vP[:, k, :],
                                rhs=aT[:, j, :],
                                start=(k == 0), stop=(k == NCHUNK - 1))
                    # normalize columns by the denominator row (row 0)
                    rec = bc_pool.tile([1, QW], F32, tag="rec")
                    nc.vector.reciprocal(out=rec, in_=ps_o[0:1, :])
                    rbc = bc_pool.tile([DP, QW], F32, tag="rbc")
                    nc.gpsimd.partition_broadcast(rbc[:, :], rec[0:1, :])
                    ot = ot_pool.tile([DP, QW], F32, tag="ot")
                    nc.vector.tensor_mul(out=ot, in0=ps_o, in1=rbc)
                    nc.sync.dma_start(
                        out=o_send[bb, 2 * w, :, :], in_=ot[:, 0:NSLOT])
                    nc.sync.dma_start(
                        out=o_send[bb, 2 * w + 1, :, :], in_=ot[:, NSLOT:QW])

                # ---- exchange: slot j -> core j (this batch only) ----
                nc.gpsimd.collective_compute(
                    "AllToAll", mybir.AluOpType.bypass,
                    replica_groups=groups,
                    ins=[o_send[bb, :, :, :]],
                    outs=[o_recv[bb, :, :, :]])

                # ---- output projection for our 256-token slice ----
                for t in range(NSLOT // P):
                    lhs = []
                    for h in range(NCORES):
                        lh = lh_pool.tile([DP, P], F32R)
                        nc.sync.dma_start(
                            out=lh, in_=o_recv[bb, h, :, t * P:(t + 1) * P].bitcast(F32R))
                        lhs.append(lh)
                    po = po_pool.tile([P, 2, 384], F32)
                    for half in range(2):
                        es = slice(half * 384, (half + 1) * 384)
                        pp = psP.tile([P, 384], F32)
                        for h in range(NCORES):
                            nc.tensor.matmul(
                                pp,
                                lhsT=lhs[h],
                                rhs=wp_sb[:, h, es],
                                start=(h == 0), stop=(h == NCORES - 1))
                        nc.scalar.mul(out=po[:, half, :], in_=pp, mul=1.0)
                    nc.sync.dma_start(
                        out=out[bb, t * P:(t + 1) * P, :],
                        in_=po.rearrange("p a b -> p (a b)"))

    nc.finalize()
    return nc


def _get_nc():
    global _CACHED_NC
    if _CACHED_NC is None:
        _CACHED_NC = _build_nc()
    return _CACHED_NC


def make_in_maps(x, W_qkv, b_qkv, W_proj, b_proj):
    x = np.asarray(x, dtype=np.float32)
    W_qkv = np.asarray(W_qkv, dtype=np.float32)
    b_qkv = np.asarray(b_qkv, dtype=np.float32)
    W_proj = np.asarray(W_proj, dtype=np.float32)
    b_proj = np.asarray(b_proj, dtype=np.float32)

    scale = 1.0 / np.sqrt(D)
    xT = np.ascontiguousarray(x.reshape(NB * SEQ, EMB).T)         # [768, 8192]
    Wr = W_qkv.reshape(EMB, NCORES, D, 3)
    br = b_qkv.reshape(NCORES, D, 3)

    wp_aug = np.zeros((NCORES, DP, EMB), dtype=np.float32)
    wp_aug[:, 1:, :] = W_proj.reshape(NCORES, D, EMB)
    wp_aug[0, 0, :] = b_proj
    wp_aug = np.ascontiguousarray(wp_aug)

    in_maps = []
    for h in range(NCORES):
        bias = np.stack(
            [br[h, :, 0] * scale, br[h, :, 1], br[h, :, 2]], axis=1)
        in_maps.append({
            "xT": xT,
            "wq": np.ascontiguousarray(Wr[:, h, :, 0] * scale),
            "wk": np.ascontiguousarray(Wr[:, h, :, 1]),
            "wv": np.ascontiguousarray(Wr[:, h, :, 2]),
            "bqkv": np.ascontiguousarray(bias),
            "wp": wp_aug,
        })
    return in_maps


def assemble(results):
    out = np.empty((NB, SEQ, EMB), dtype=np.float32)
    for c in range(NCORES):
        out[:, c * NSLOT:(c + 1) * NSLOT, :] = results[c]["out"]
    return out


def kernel(x, W_qkv, b_qkv, W_proj, b_proj):
    nc = _get_nc()
    in_maps = make_in_maps(x, W_qkv, b_qkv, W_proj, b_proj)
    r = run_bass_kernel_spmd(nc, in_maps, core_ids=list(range(NCORES)))
    return assemble(r.results)
